# revision 2
# baseline (speedup 1.0000x reference)
"""Trainium2 Bass kernel for T5-style cross-attention, sharded over 8 NeuronCores.

Sharding: tensor-parallel over heads (16 heads -> 2 per core). Each core
computes Q/K/V projections for its 2 heads (full batch), flash-style
attention with additive position bias, and a partial output projection
against its row-slice of Wo. The host sums the 8 partial outputs
(the unshard step for a row-sharded Wo).

Kernel-internal layout is fully transposed (S^T = [k, q] tiles) so the
softmax denominator comes out of the PE via a ones-column appended to V,
and no on-chip transposes are needed. The host pre-transposes x/encoding/
bias once so every DMA is a contiguous natural load.
"""

import sys

try:
    import concourse.bass as bass
except ImportError:
    sys.path.insert(0, "/opt/trn_rl_repo")
    import concourse.bass as bass

import numpy as np

import concourse.mybir as mybir
from concourse import bacc
from concourse.tile import TileContext
from concourse.bass_utils import run_bass_kernel_spmd

F32 = mybir.dt.float32
F32R = mybir.dt.float32r

# Problem sizes (hardcoded per spec)
B, NQ, NKV = 4, 2048, 2048
D_MODEL, N_HEADS, D_K = 1024, 16, 64
N_CORES = 8
HPC = N_HEADS // N_CORES          # heads per core = 2
DH = HPC * D_K                    # 128 partition rows of per-core head dims

QW = 512                          # q window (matmul free dim)
KT = 128                          # k tile (partition dim of S^T)
MT = 128                          # model-dim contraction tile
KG = 2                            # k tiles per exp group ([128, KG*QW] psum)


def build_kernel(b=B, nq=NQ, nkv=NKV, d_model=D_MODEL):
    nc = bacc.Bacc("TRN2", target_bir_lowering=False, debug=False,
                   num_devices=N_CORES)

    xT = nc.dram_tensor("xT", [b, d_model, nq], F32R, kind="ExternalInput")
    encT = nc.dram_tensor("encT", [b, d_model, nkv], F32R, kind="ExternalInput")
    biasT = nc.dram_tensor("biasT", [HPC, nkv, nq], F32R, kind="ExternalInput")
    wq = nc.dram_tensor("wq", [d_model, DH], F32R, kind="ExternalInput")
    wk = nc.dram_tensor("wk", [d_model, DH], F32R, kind="ExternalInput")
    wv = nc.dram_tensor("wv", [d_model, DH], F32R, kind="ExternalInput")
    wo = nc.dram_tensor("wo", [DH, d_model], F32R, kind="ExternalInput")
    consts = nc.dram_tensor("consts", [128, 129], F32R, kind="ExternalInput")
    out = nc.dram_tensor("out", [b, nq, d_model], F32, kind="ExternalOutput")

    n_m = d_model // MT           # model-dim tiles (8)
    n_qw = nq // QW               # q windows (4)
    n_kw = nkv // QW              # k windows for proj (4)
    n_kt = nkv // KT              # k tiles (16)
    n_kg = n_kt // KG             # exp groups (8)
    n_e = d_model // QW           # output column halves (2)

    with TileContext(nc) as tc:
        with (
            tc.tile_pool(name="cst", bufs=1) as cst,
            tc.tile_pool(name="wpool", bufs=1) as wpool,
            tc.tile_pool(name="qkv", bufs=1) as qkv,
            tc.tile_pool(name="actst", bufs=4) as actst,
            tc.tile_pool(name="sbias", bufs=3) as sbias,
            tc.tile_pool(name="sattn", bufs=3) as sattn,
            tc.tile_pool(name="sctx", bufs=2 * b) as sctx,
            tc.tile_pool(name="sout", bufs=3) as sout,
            tc.tile_pool(name="ssmall", bufs=6) as ssmall,
            tc.tile_pool(name="psbig", bufs=2, space="PSUM") as psbig,
            tc.tile_pool(name="pssmall", bufs=4, space="PSUM") as pssmall,
        ):
            # ---- constants & weights ----
            ident = cst.tile([128, 128], F32R, tag="ident")
            nc.sync.dma_start(out=ident, in_=consts[:, 0:128])
            ones_col = cst.tile([128, 1], F32R, tag="ones")
            nc.sync.dma_start(out=ones_col, in_=consts[:, 128:129])

            wq_sb = wpool.tile([128, n_m * DH], F32R, tag="wq")
            wk_sb = wpool.tile([128, n_m * DH], F32R, tag="wk")
            wv_sb = wpool.tile([128, n_m * DH], F32R, tag="wv")
            for m in range(n_m):
                nc.sync.dma_start(out=wq_sb[:, m * DH:(m + 1) * DH],
                                  in_=wq[m * MT:(m + 1) * MT, :])
                nc.sync.dma_start(out=wk_sb[:, m * DH:(m + 1) * DH],
                                  in_=wk[m * MT:(m + 1) * MT, :])
                nc.sync.dma_start(out=wv_sb[:, m * DH:(m + 1) * DH],
                                  in_=wv[m * MT:(m + 1) * MT, :])
            wo_sb = wpool.tile([128, d_model], F32R, tag="wo")
            nc.sync.dma_start(out=wo_sb, in_=wo[:, :])

            # ---- phase A: projections ----
            # Q^T / K^T: [128 (2h x 64d), b*n] f32r, b-major free dim
            qT_sb = qkv.tile([128, b * nq], F32R, tag="qT")
            kT_sb = qkv.tile([128, b * nkv], F32R, tag="kT")
            # Vones tiles [128(k), 65] per (b, ktile, h)
            vones = {}
            for bi in range(b):
                for kt in range(n_kt):
                    for h in range(HPC):
                        vones[(bi, kt, h)] = qkv.tile(
                            [128, D_K + 1], F32R, tag=f"v_{bi}_{kt}_{h}",
                            name=f"v_{bi}_{kt}_{h}")

            for bi in range(b):
                # Q^T projection over q windows
                for qw in range(n_qw):
                    q_ps = psbig.tile([128, QW], F32, tag="big")
                    for m in range(n_m):
                        xt = actst.tile([128, QW], F32R, tag="actst")
                        nc.sync.dma_start(
                            out=xt,
                            in_=xT[bi, m * MT:(m + 1) * MT,
                                   qw * QW:(qw + 1) * QW])
                        nc.tensor.matmul(q_ps,
                                         wq_sb[:, m * DH:(m + 1) * DH], xt,
                                         start=(m == 0), stop=(m == n_m - 1))
                    nc.vector.tensor_copy(
                        qT_sb[:, bi * nq + qw * QW: bi * nq + (qw + 1) * QW],
                        q_ps)
                # K^T and V projections over k windows
                for kw in range(n_kw):
                    k_ps = psbig.tile([128, QW], F32, tag="big")
                    v_ps = [pssmall.tile([128, DH], F32, tag="small",
                                         name=f"vps_{bi}_{kw}_{s}")
                            for s in range(QW // KT)]
                    for m in range(n_m):
                        et = actst.tile([128, QW], F32R, tag="actst")
                        nc.sync.dma_start(
                            out=et,
                            in_=encT[bi, m * MT:(m + 1) * MT,
                                     kw * QW:(kw + 1) * QW])
                        nc.tensor.matmul(k_ps,
                                         wk_sb[:, m * DH:(m + 1) * DH], et,
                                         start=(m == 0), stop=(m == n_m - 1))
                        for s in range(QW // KT):
                            nc.tensor.matmul(
                                v_ps[s],
                                et[:, s * KT:(s + 1) * KT],
                                wv_sb[:, m * DH:(m + 1) * DH],
                                start=(m == 0), stop=(m == n_m - 1))
                    nc.vector.tensor_copy(
                        kT_sb[:, bi * nkv + kw * QW: bi * nkv + (kw + 1) * QW],
                        k_ps)
                    for s in range(QW // KT):
                        kt = kw * (QW // KT) + s
                        for h in range(HPC):
                            vt = vones[(bi, kt, h)]
                            nc.vector.tensor_copy(
                                vt[:, 0:D_K],
                                v_ps[s][:, h * D_K:(h + 1) * D_K])
                            nc.vector.tensor_copy(vt[:, D_K:D_K + 1], ones_col)

            # ---- phase B: flash attention + output projection ----
            for qw in range(n_qw):
                ctx_t = [sctx.tile([128, QW], F32R, tag="ctx",
                                   name=f"ctx_{qw}_{bi}")
                         for bi in range(b)]
                for h in range(HPC):
                    hp = h * D_K
                    u_t = [pssmall.tile([D_K + 1, QW], F32, tag="small",
                                        name=f"u_{qw}_{h}_{bi}")
                           for bi in range(b)]
                    for kg in range(n_kg):
                        bias_sb = sbias.tile([128, KG * QW], F32R, tag="bias")
                        nc.sync.dma_start(
                            out=bias_sb.rearrange("p (t q) -> p t q", t=KG),
                            in_=biasT[h, kg * KG * KT:(kg + 1) * KG * KT,
                                      qw * QW:(qw + 1) * QW]
                            .rearrange("(t p) q -> p t q", p=KT))
                        for bi in range(b):
                            s_ps = psbig.tile([128, KG * QW], F32, tag="big")
                            for j in range(KG):
                                kt = kg * KG + j
                                sl = s_ps[:, j * QW:(j + 1) * QW]
                                nc.tensor.matmul(
                                    sl,
                                    kT_sb[hp:hp + D_K,
                                          bi * nkv + kt * KT:
                                          bi * nkv + (kt + 1) * KT],
                                    qT_sb[hp:hp + D_K,
                                          bi * nq + qw * QW:
                                          bi * nq + (qw + 1) * QW],
                                    start=True, stop=False)
                                nc.tensor.matmul(
                                    sl, ident, bias_sb[:, j * QW:(j + 1) * QW],
                                    start=False, stop=True)
                            attn = sattn.tile([128, KG * QW], F32R, tag="attn")
                            nc.scalar.activation(
                                attn, s_ps, mybir.ActivationFunctionType.Exp)
                            for j in range(KG):
                                kt = kg * KG + j
                                nc.tensor.matmul(
                                    u_t[bi],
                                    vones[(bi, kt, h)],
                                    attn[:, j * QW:(j + 1) * QW],
                                    start=(kt == 0), stop=(kt == n_kt - 1),
                                    skip_group_check=True)
                    for bi in range(b):
                        recip = ssmall.tile([1, QW], F32, tag="recip")
                        nc.vector.reciprocal(recip, u_t[bi][D_K:D_K + 1, :])
                        rb = ssmall.tile([D_K, QW], F32, tag="rb")
                        nc.gpsimd.partition_broadcast(rb, recip)
                        with nc.allow_low_precision(reason="fp32r ctx for PE"):
                            nc.vector.tensor_mul(
                                ctx_t[bi][hp:hp + D_K, :],
                                u_t[bi][0:D_K, :], rb)
                # Wo projection for this q window
                for bi in range(b):
                    for qs in range(QW // 128):
                        q0 = qw * QW + qs * 128
                        for e in range(n_e):
                            o_ps = psbig.tile([128, QW], F32, tag="big")
                            nc.tensor.matmul(
                                o_ps,
                                ctx_t[bi][:, qs * 128:(qs + 1) * 128],
                                wo_sb[:, e * QW:(e + 1) * QW],
                                start=True, stop=True)
                            o_sb = sout.tile([128, QW], F32, tag="out")
                            nc.vector.tensor_copy(o_sb, o_ps)
                            nc.sync.dma_start(
                                out=out[bi, q0:q0 + 128,
                                        e * QW:(e + 1) * QW],
                                in_=o_sb)
    nc.compile()
    return nc


_NC_CACHE = {}


def _get_nc():
    if "nc" not in _NC_CACHE:
        _NC_CACHE["nc"] = build_kernel()
    return _NC_CACHE["nc"]


def kernel(x, encoding, position_bias, Wq, Wk, Wv, Wo):
    x = np.asarray(x, np.float32)
    encoding = np.asarray(encoding, np.float32)
    position_bias = np.asarray(position_bias, np.float32)
    Wq = np.asarray(Wq, np.float32)
    Wk = np.asarray(Wk, np.float32)
    Wv = np.asarray(Wv, np.float32)
    Wo = np.asarray(Wo, np.float32)

    xT = np.ascontiguousarray(x.transpose(0, 2, 1))
    encT = np.ascontiguousarray(encoding.transpose(0, 2, 1))
    consts = np.concatenate(
        [np.eye(128, dtype=np.float32), np.ones((128, 1), np.float32)], axis=1)
    consts = np.ascontiguousarray(consts)

    in_maps = []
    for c in range(N_CORES):
        h0 = c * HPC
        in_maps.append({
            "xT": xT,
            "encT": encT,
            "biasT": np.ascontiguousarray(
                position_bias[0, h0:h0 + HPC].transpose(0, 2, 1)),
            "wq": np.ascontiguousarray(Wq[:, h0 * D_K:(h0 + HPC) * D_K]),
            "wk": np.ascontiguousarray(Wk[:, h0 * D_K:(h0 + HPC) * D_K]),
            "wv": np.ascontiguousarray(Wv[:, h0 * D_K:(h0 + HPC) * D_K]),
            "wo": np.ascontiguousarray(Wo[h0 * D_K:(h0 + HPC) * D_K, :]),
            "consts": consts,
        })

    nc = _get_nc()
    res = run_bass_kernel_spmd(nc, in_maps, list(range(N_CORES)))
    acc = res.results[0]["out"].astype(np.float32)
    for c in range(1, N_CORES):
        acc = acc + res.results[c]["out"]
    return acc
